# revision 3
# baseline (speedup 1.0000x reference)
"""DCNv2 PSRoI pooling (deformable) Trainium2 Bass kernel.

Problem: input [4,256,128,128] f32, rois [1024,5], offset [1024,2,7,7]
-> out [1024,256,7,7].  Sharded data-parallel over RoIs across 8 cores
(128 RoIs/core); the channel-last feature map is replicated (in bf16).

Per (roi, bin): the 4x4 deformable bilinear samples live inside a
6-row x 7-col feature patch whose bilinear weights collapse to an outer
product wy[6] (x) wx[7].  Each core:
  1. computes sample geometry + weights on DVE (partition = roi),
  2. dma_gather's the 6 row-segments (7 px * 256 ch = 3.5KB bf16) per
     bin, indexed by pixel-pairs (1KB units) so indices fit int16,
  3. reduces: y on DVE as two bf16 3-row MAC chains (precision), the
     row-combine + x-MAC in f32 on GpSimd,
  4. writes [128, 256] f32 per bin to DRAM.

Self-contained: numpy + ml_dtypes + concourse.
"""

from contextlib import ExitStack

import numpy as np
import ml_dtypes

import concourse.bass as bass
import concourse.tile as tile
from concourse import bacc, mybir
from concourse.bass_utils import run_bass_kernel_spmd

F32 = mybir.dt.float32
BF16 = mybir.dt.bfloat16
I32 = mybir.dt.int32
I16 = mybir.dt.int16
OP = mybir.AluOpType

N_CORES = 8
RPC = 128                 # rois per core
NB = 49                   # bins per roi
C = 256
B, H, W = 4, 128, 128
SPP = 4
S = 0.0625
TRANS_STD = 0.1

PAIR = 512                # elems per index unit (2 pixels)
SEG = 7 * C               # 1792 elems per gathered segment (7 px)
NROWS = 6
NCOLS = 7
FEAT_ELEMS = B * H * W * C
PAD_ELEMS = 2048
MAGIC = 8388608.0         # 2**23
P = 128


def _emit(ctx: ExitStack, tc: "tile.TileContext", feat_h, rois_h, offs_h, out_h,
          n_repeats: int = 1, mac_mode: str = "full"):
    nc = tc.nc
    V = nc.vector
    G = nc.gpsimd

    setup = ctx.enter_context(tc.tile_pool(name="setup", bufs=1))
    patches = ctx.enter_context(tc.tile_pool(name="patches", bufs=5))
    accs = ctx.enter_context(tc.tile_pool(name="accs", bufs=2))
    outs = ctx.enter_context(tc.tile_pool(name="outs", bufs=3))

    def st(shape, dtype=F32, tag=None):
        return setup.tile(shape, dtype, tag=tag, name=tag)

    def ts(out, in0, s1, op0, s2=None, op1=None):
        if op1 is None:
            return V.tensor_scalar(out, in0, s1, None, op0)
        return V.tensor_scalar(out, in0, s1, s2, op0, op1)

    def tt(out, in0, in1, op):
        return V.tensor_tensor(out=out, in0=in0, in1=in1, op=op)

    def b3(t, n):
        """[P,NB] tile -> broadcast AP [P,NB,n]."""
        return t[:, :].unsqueeze(2).to_broadcast([P, NB, n])

    def r3(t, n):
        """[P,NB*n] tile -> [P,NB,n] view."""
        return t[:, :].rearrange("p (b j) -> p b j", b=NB, j=n)

    # ---- load rois / offsets ----
    rt = st([P, 5], tag="rt")
    ot = st([P, 2 * NB], tag="ot")
    nc.sync.dma_start(out=rt[:, :], in_=rois_h[:, :])
    nc.sync.dma_start(out=ot[:, :], in_=offs_h[:, :])

    # ---- iotas (gpsimd int32 -> f32 copies) ----
    def iota_f32(pattern, n, tag):
        it = st([P, n], I32, tag=tag + "_i")
        nc.gpsimd.iota(it[:, :], pattern, base=0, channel_multiplier=0)
        ft = st([P, n], F32, tag=tag)
        V.tensor_copy(out=ft[:, :], in_=it[:, :])
        return ft

    iot_pw = iota_f32([[0, 7], [1, 7]], NB, "iot_pw")      # k % 7
    iot_ph = iota_f32([[1, 7], [0, 7]], NB, "iot_ph")      # k // 7
    iot_x7 = iota_f32([[0, NB], [1, NCOLS]], NB * NCOLS, "iot_x7")
    iot_y6 = iota_f32([[0, NB], [1, NROWS]], NB * NROWS, "iot_y6")
    iot_g64 = iota_f32([[0, NB], [64, NROWS]], NB * NROWS, "iot_g64")

    # ---- per-roi columns [P, 1] ----
    cols = st([P, 12], tag="cols")
    b_c = cols[:, 0:1]
    x1_c = cols[:, 1:2]
    y1_c = cols[:, 2:3]
    rw_c = cols[:, 3:4]
    rh_c = cols[:, 4:5]
    bw_c = cols[:, 5:6]
    bh_c = cols[:, 6:7]
    sw_c = cols[:, 7:8]
    sh_c = cols[:, 8:9]
    bb_c = cols[:, 9:10]   # b * 8192
    t0 = st([P, 1], tag="t0")
    t1 = st([P, 1], tag="t1")
    t2 = st([P, 1], tag="t2")

    V.tensor_copy(out=b_c, in_=rt[:, 0:1])
    ts(bb_c, b_c, 8192.0, OP.mult)

    def rnd_col(out, src_ap):
        # out = floor(src + 0.5)
        ts(t0, src_ap, 0.5, OP.add)
        ts(t1, t0, MAGIC, OP.add, MAGIC, OP.subtract)
        tt(t2, t1, t0, OP.is_gt)
        tt(out, t1, t2, OP.subtract)

    # x1 = round(r1)*S - 0.5 ; x2 = (round(r3)+1)*S - 0.5 = round(r3)*S - (0.5-S)
    rnd_col(t0, rt[:, 1:2])
    ts(x1_c, t0, S, OP.mult, 0.5, OP.subtract)
    rnd_col(t0, rt[:, 3:4])
    ts(t1, t0, S, OP.mult, 0.5 - S, OP.subtract)
    tt(rw_c, t1, x1_c, OP.subtract)
    ts(rw_c, rw_c, 0.1, OP.max)

    rnd_col(t0, rt[:, 2:3])
    ts(y1_c, t0, S, OP.mult, 0.5, OP.subtract)
    rnd_col(t0, rt[:, 4:5])
    ts(t1, t0, S, OP.mult, 0.5 - S, OP.subtract)
    tt(rh_c, t1, y1_c, OP.subtract)
    ts(rh_c, rh_c, 0.1, OP.max)

    # ---- exact f32 division by 7 (verified bit-exact over the quantized
    # domain of rw/rh = max(m/16, 0.1)): m=16x; k=floor(m/7); s=m-7k;
    # hi/lo split table of s/7; Fast2Sum(k, hi) + lo; scale by 1/16 ----
    R7 = float(np.float32(1.0) / np.float32(7.0))
    C01 = float(np.float32(0.1) / np.float32(7.0))
    dv_m = st([P, 1], tag="dv_m")
    dv_k = st([P, 1], tag="dv_k")
    dv_s = st([P, 1], tag="dv_s")
    dv_hi = st([P, 1], tag="dv_hi")
    dv_lo = st([P, 1], tag="dv_lo")
    dv_t = st([P, 1], tag="dv_t")
    dv_z = st([P, 1], tag="dv_z")
    dv_e = st([P, 1], tag="dv_e")

    def exact_div7(out, x_col):
        ts(dv_m, x_col, 16.0, OP.mult)
        ts(dv_t, dv_m, R7, OP.mult)
        ts(dv_k, dv_t, MAGIC, OP.add, MAGIC, OP.subtract)
        tt(dv_z, dv_k, dv_t, OP.is_gt)
        tt(dv_k, dv_k, dv_z, OP.subtract)
        V.scalar_tensor_tensor(out=dv_s, in0=dv_k, scalar=-7.0, in1=dv_m,
                               op0=OP.mult, op1=OP.add)
        V.memset(dv_hi[:, :], 0.0)
        V.memset(dv_lo[:, :], 0.0)
        for sv in range(1, 7):
            th = float(np.float32(sv / 7.0))
            tl = float(np.float32(sv / 7.0 - np.float32(sv / 7.0)))
            ts(dv_e, dv_s, float(sv), OP.is_equal)
            V.scalar_tensor_tensor(out=dv_hi, in0=dv_e, scalar=th, in1=dv_hi,
                                   op0=OP.mult, op1=OP.add)
            V.scalar_tensor_tensor(out=dv_lo, in0=dv_e, scalar=tl, in1=dv_lo,
                                   op0=OP.mult, op1=OP.add)
        tt(dv_t, dv_k, dv_hi, OP.add)
        tt(dv_z, dv_t, dv_k, OP.subtract)
        tt(dv_e, dv_hi, dv_z, OP.subtract)
        tt(dv_e, dv_e, dv_lo, OP.add)
        tt(dv_t, dv_t, dv_e, OP.add)
        ts(out, dv_t, 0.0625, OP.mult)
        # fix x == 0.1 exactly
        ts(dv_e, x_col, 0.1, OP.is_equal)
        ts(dv_z, out, -1.0, OP.mult, C01, OP.add)  # C01 - out
        tt(dv_z, dv_z, dv_e, OP.mult)
        tt(out, out, dv_z, OP.add)

    exact_div7(bw_c, rw_c)
    exact_div7(bh_c, rh_c)
    ts(sw_c, bw_c, 0.25, OP.mult)
    ts(sh_c, bh_c, 0.25, OP.mult)

    # ---- per-bin starts [P, NB] ----
    txt = st([P, NB], tag="txt")
    tyt = st([P, NB], tag="tyt")
    wstart = st([P, NB], tag="wstart")
    hstart = st([P, NB], tag="hstart")
    u0 = st([P, NB], tag="u0")
    ts(txt, ot[:, 0:NB], TRANS_STD, OP.mult)
    ts(tyt, ot[:, NB:2 * NB], TRANS_STD, OP.mult)
    # wstart = pw*bw + x1 + tx*rw
    ts(u0, iot_pw[:, :], bw_c, OP.mult)
    V.scalar_tensor_tensor(out=wstart[:, :], in0=txt[:, :], scalar=rw_c,
                           in1=u0[:, :], op0=OP.mult, op1=OP.add)
    ts(wstart, wstart, x1_c, OP.add)
    ts(u0, iot_ph[:, :], bh_c, OP.mult)
    V.scalar_tensor_tensor(out=hstart[:, :], in0=tyt[:, :], scalar=rh_c,
                           in1=u0[:, :], op0=OP.mult, op1=OP.add)
    ts(hstart, hstart, y1_c, OP.add)

    # ---- per-sample floor / contributions, stashed per sample ----
    def axis_samples(start_t, step_c, limit, name):
        cnt = st([P, NB], tag=f"{name}_cnt")
        wv = st([P, NB], tag=f"{name}_wv")
        vx = st([P, NB], tag=f"{name}_vx")
        wc = st([P, NB], tag=f"{name}_wc")
        g0 = st([P, NB], tag=f"{name}_g0")
        stp = st([P, 1], tag=f"{name}_stp")
        fls, c0s, c1s = [], [], []
        for s in range(SPP):
            fl = st([P, NB], tag=f"{name}_fl{s}")
            c0 = st([P, NB], tag=f"{name}_c0{s}")
            c1 = st([P, NB], tag=f"{name}_c1{s}")
            if s == 0:
                V.tensor_copy(out=wv[:, :], in_=start_t[:, :])
            else:
                ts(stp, step_c, float(s), OP.mult)
                ts(wv, start_t[:, :], stp, OP.add)
            ts(g0, wv, -0.5, OP.is_ge)
            ts(vx, wv, limit + 0.5, OP.is_le)
            tt(vx, vx, g0, OP.mult)
            if s == 0:
                V.tensor_copy(out=cnt[:, :], in_=vx[:, :])
            else:
                tt(cnt, cnt, vx, OP.add)
            ts(wc, wv, 0.0, OP.max, float(limit), OP.min)
            ts(fl, wc, MAGIC, OP.add, MAGIC, OP.subtract)
            tt(g0, fl, wc, OP.is_gt)
            tt(fl, fl, g0, OP.subtract)
            tt(wc, wc, fl, OP.subtract)          # wc now holds dx
            tt(c1, vx, wc, OP.mult)
            tt(c0, vx, c1, OP.subtract)
            fls.append(fl)
            c0s.append(c0)
            c1s.append(c1)
        return fls, c0s, c1s, cnt

    def accum_weights(wsum, eq, eqc, rel, c0, c1, iot_n, n, first):
        w3 = r3(wsum, n)
        e3 = r3(eq, n)
        ec3 = r3(eqc, n)
        tt(e3, b3(rel, n), r3(iot_n, n), OP.is_equal)
        if first:
            tt(w3, e3, b3(c0, n), OP.mult)
        else:
            tt(ec3, e3, b3(c0, n), OP.mult)
            tt(w3, w3, ec3, OP.add)
        # shifted c1 into slots 1..n-1
        cm = c1[:, :].unsqueeze(2).to_broadcast([P, NB, n - 1])
        tt(ec3[:, :, 0:n - 1], e3[:, :, 0:n - 1], cm, OP.mult)
        tt(w3[:, :, 1:n], w3[:, :, 1:n], ec3[:, :, 0:n - 1], OP.add)

    yfls, yc0s, yc1s, cnty = axis_samples(hstart, sh_c, H - 1, "y")
    xfls, xc0s, xc1s, cntx = axis_samples(wstart, sw_c, W - 1, "x")

    yb = yfls[0]                                   # row base (do not modify)

    # even-aligned x base: xe = 2*floor(xb/2); xh = floor(xb/2) (pair units)
    xb = xfls[0]
    xe = st([P, NB], tag="xe")
    xh = st([P, NB], tag="xh")
    xg = st([P, NB], tag="xg")
    ts(xh, xb[:, :], 0.5, OP.mult)
    ts(xe, xh, MAGIC, OP.add, MAGIC, OP.subtract)
    tt(xg, xe, xh, OP.is_gt)
    tt(xh, xe, xg, OP.subtract)                    # floor(xb/2)
    ts(xe, xh, 2.0, OP.mult)

    # ---- accumulate outer-product weights ----
    wy = st([P, NB * NROWS], tag="wy")
    wx = st([P, NB * NCOLS], tag="wx")
    yeq = st([P, NB * NROWS], tag="yeq")
    yeqc = st([P, NB * NROWS], tag="yeqc")
    xeq = st([P, NB * NCOLS], tag="xeq")
    xeqc = st([P, NB * NCOLS], tag="xeqc")
    yrel = st([P, NB], tag="yrel")
    xrel = st([P, NB], tag="xrel")
    for s in range(SPP):
        tt(yrel, yfls[s], yb, OP.subtract)
        accum_weights(wy, yeq, yeqc, yrel, yc0s[s], yc1s[s], iot_y6, NROWS, s == 0)
    for s in range(SPP):
        tt(xrel, xfls[s], xe, OP.subtract)
        accum_weights(wx, xeq, xeqc, xrel, xc0s[s], xc1s[s], iot_x7, NCOLS, s == 0)

    # ---- wy /= max(cnt,1) ----
    cnt = st([P, NB], tag="cnt")
    rec = st([P, NB], tag="rec")
    tt(cnt, cntx, cnty, OP.mult)
    ts(cnt, cnt, 1.0, OP.max)
    V.reciprocal(out=rec[:, :], in_=cnt[:, :])
    tt(r3(wy, NROWS), r3(wy, NROWS), b3(rec, NROWS), OP.mult)

    # ---- pair indices ----
    pairf = st([P, NB * NROWS], tag="pairf")
    basei = st([P, NB], tag="basei")
    V.scalar_tensor_tensor(out=basei[:, :], in0=yb[:, :], scalar=64.0,
                           in1=xh[:, :], op0=OP.mult, op1=OP.add)
    ts(basei, basei, bb_c, OP.add)
    tt(r3(pairf, NROWS), b3(basei, NROWS), r3(iot_g64, NROWS), OP.add)
    ts(pairf, pairf, 32767.0, OP.min)
    pii = st([P, NB * NROWS], I32, tag="pii")
    V.tensor_copy(out=pii[:, :], in_=pairf[:, :])
    pi16 = st([P, NB * NROWS], I16, tag="pi16")
    V.tensor_copy(out=pi16[:, :], in_=pii[:, :])

    # ---- idx layout: pos [j%16, j//16] for j = g*128 + p, replicated x8 ----
    # step 1: partition blocks 16h..16h+16 of pi16 -> partitions 0..16 of tmp
    # (engine ops can only start at partition 0/32/64/96, DMA can start anywhere)
    idxtmp = st([16, NB * NROWS * 8], I16, tag="idxtmp")  # [16, h*294+bin*6+g]
    for h in range(8):
        nc.sync.dma_start(
            out=idxtmp[0:16, h * (NB * NROWS):(h + 1) * (NB * NROWS)],
            in_=pi16[16 * h:16 * (h + 1), :],
        )
    # step 2: strided rearrange (h*294 + bin*6 + g) -> (bin*48 + g*8 + h) on DVE
    idx16 = st([P, NB * NROWS * 8], I16, tag="idx16")   # [128, 2352]
    i4 = idx16[:, :].rearrange("p (b g h) -> p b g h", b=NB, g=NROWS, h=8)
    tmp4 = bass.AP(idxtmp.tensor, idxtmp.offset,
                   [idxtmp.ap[0], [6, NB], [1, NROWS], [NB * NROWS, 8]])
    V.tensor_copy(out=i4[0:16, :, :, :], in_=tmp4)
    # step 3: replicate to all 128 partitions by doubling DMAs
    nc.sync.dma_start(out=idx16[16:32, :], in_=idx16[0:16, :])
    nc.sync.dma_start(out=idx16[32:64, :], in_=idx16[0:32, :])
    nc.sync.dma_start(out=idx16[64:128, :], in_=idx16[0:64, :])

    # ---- gather + reduce per bin ----
    feat_ap = bass.AP(feat_h, 0, [[PAIR, 32768], [1, SEG]])
    nreg = nc.gpsimd.to_reg(NROWS * P)
    for bin_ in [b for _ in range(n_repeats) for b in range(NB)]:
        patch = patches.tile([P, NROWS, SEG], BF16, tag="patch", name="patch")
        nc.gpsimd.dma_gather(
            patch[:, :, :],
            feat_ap,
            idx16[:, bin_ * 48:(bin_ + 1) * 48],
            num_idxs=NROWS * P,
            num_idxs_reg=nreg,
            elem_size=SEG,
            elem_step=PAIR,
            queue_num=bin_ % 2,
        )
        ob = outs.tile([P, C], F32, tag="ob", name="ob")
        if mac_mode == "full":
            # y-reduction: two 3-row bf16 MAC chains on DVE
            accA = accs.tile([P, SEG], BF16, tag="accA", name="accA")
            accB = accs.tile([P, SEG], BF16, tag="accB", name="accB")
            for half, acc in ((0, accA), (1, accB)):
                for j in range(3):
                    g = half * 3 + j
                    wcol = wy[:, bin_ * NROWS + g:bin_ * NROWS + g + 1]
                    if j == 0:
                        V.tensor_scalar(acc[:, :], patch[:, g, :], wcol, None,
                                        OP.mult)
                    else:
                        V.scalar_tensor_tensor(out=acc[:, :], in0=patch[:, g, :],
                                               scalar=wcol, in1=acc[:, :],
                                               op0=OP.mult, op1=OP.add)
            # combine halves in f32 on GpSimd (per-partition-scalar ops are
            # DVE-only; plain tensor_tensor is legal on Pool)
            accT = accs.tile([P, SEG], F32, tag="accT", name="accT")
            G.tensor_tensor(out=accT[:, :], in0=accA[:, :], in1=accB[:, :],
                            op=OP.add)
            # x-reduction in f32 on DVE
            for x in range(NCOLS):
                wcol = wx[:, bin_ * NCOLS + x:bin_ * NCOLS + x + 1]
                if x == 0:
                    V.tensor_scalar(ob[:, :], accT[:, 0:C], wcol, None, OP.mult)
                else:
                    V.scalar_tensor_tensor(out=ob[:, :],
                                           in0=accT[:, x * C:(x + 1) * C],
                                           scalar=wcol, in1=ob[:, :],
                                           op0=OP.mult, op1=OP.add)
        else:
            # gather-floor probe: one cheap consumer per patch
            V.tensor_scalar(ob[:, :], patch[:, 0, 0:C],
                            wy[:, bin_ * NROWS:bin_ * NROWS + 1], None, OP.mult)
        nc.sync.dma_start(out=out_h[:, bin_, :], in_=ob[:, :])


_CACHE = {}


def build_program(n_repeats: int = 1, mac_mode: str = "full"):
    key = ("nc", n_repeats, mac_mode)
    if key in _CACHE:
        return _CACHE[key]
    nc = bacc.Bacc(num_swdge_queues=2)
    feat_h = nc.declare_dram_parameter("feat", [FEAT_ELEMS + PAD_ELEMS], BF16,
                                       isOutput=False)
    rois_h = nc.declare_dram_parameter("rois", [RPC, 5], F32, isOutput=False)
    offs_h = nc.declare_dram_parameter("offs", [RPC, 2 * NB], F32, isOutput=False)
    out_h = nc.declare_dram_parameter("out", [RPC, NB, C], F32, isOutput=True)
    with tile.TileContext(nc) as tc, ExitStack() as ctx:
        _emit(ctx, tc, feat_h, rois_h, offs_h, out_h, n_repeats=n_repeats,
              mac_mode=mac_mode)
    nc.compile()
    _CACHE[key] = nc
    return nc


def make_in_maps(input, rois, offset):
    inp = np.ascontiguousarray(np.asarray(input, np.float32))
    rois = np.asarray(rois, np.float32)
    offset = np.asarray(offset, np.float32)
    featflat = np.concatenate(
        [inp.transpose(0, 2, 3, 1).reshape(-1), np.zeros(PAD_ELEMS, np.float32)]
    ).astype(ml_dtypes.bfloat16)
    in_maps = []
    for c in range(N_CORES):
        sl = slice(c * RPC, (c + 1) * RPC)
        in_maps.append({
            "feat": featflat,
            "rois": np.ascontiguousarray(rois[sl]),
            "offs": np.ascontiguousarray(offset[sl].reshape(RPC, 2 * NB)),
        })
    return in_maps


def kernel(input, rois, offset, _trace=False, _trace_kwargs=None):
    nc = build_program()
    in_maps = make_in_maps(input, rois, offset)
    res = run_bass_kernel_spmd(
        nc, in_maps, list(range(N_CORES)),
        trace=_trace, **(_trace_kwargs or {}),
    )
    outs_np = [res.results[c]["out"] for c in range(N_CORES)]
    full = np.concatenate(outs_np, axis=0)          # [1024, 49, 256]
    out = full.reshape(1024, 7, 7, 256).transpose(0, 3, 1, 2)
    if _trace:
        return np.ascontiguousarray(out), res
    return np.ascontiguousarray(out)


# revision 5
# speedup vs baseline: 2.5327x; 2.5327x over previous
"""DCNv2 PSRoI pooling (deformable) Trainium2 Bass kernel.

Problem: input [4,256,128,128] f32, rois [1024,5], offset [1024,2,7,7]
-> out [1024,256,7,7].  Sharded data-parallel over RoIs across 8 cores
(128 RoIs/core); the channel-last feature map is replicated (in bf16).

Per (roi, bin): the 4x4 deformable bilinear samples live inside a
6-row x 7-col feature patch whose bilinear weights collapse to an outer
product wy[6] (x) wx[7].  Each core:
  1. computes sample geometry + weights on DVE (partition = roi),
  2. dma_gather's the 6 row-segments (7 px * 256 ch = 3.5KB bf16) per
     bin, indexed by pixel-pairs (1KB units) so indices fit int16,
  3. reduces: y on DVE as two bf16 3-row MAC chains (precision), the
     row-combine + x-MAC in f32 on GpSimd,
  4. writes [128, 256] f32 per bin to DRAM.

Self-contained: numpy + ml_dtypes + concourse.
"""

from contextlib import ExitStack

import numpy as np
import ml_dtypes

import concourse.bass as bass
import concourse.tile as tile
from concourse import bacc, mybir
from concourse.bass_utils import run_bass_kernel_spmd

F32 = mybir.dt.float32
BF16 = mybir.dt.bfloat16
I32 = mybir.dt.int32
I16 = mybir.dt.int16
OP = mybir.AluOpType

N_CORES = 8
RPC = 128                 # rois per core
NB = 49                   # bins per roi
C = 256
B, H, W = 4, 128, 128
SPP = 4
S = 0.0625
TRANS_STD = 0.1

PAIR = 512                # elems per index unit (2 pixels)
SEG = 7 * C               # 1792 elems per gathered segment (7 px)
NROWS = 6
NCOLS = 7
FEAT_ELEMS = B * H * W * C
PAD_ELEMS = 2048
MAGIC = 8388608.0         # 2**23
P = 128


def _emit(ctx: ExitStack, tc: "tile.TileContext", feat_h, rois_h, offs_h, out_h,
          n_repeats: int = 1, mac_mode: str = "full"):
    nc = tc.nc
    V = nc.vector
    G = nc.gpsimd

    setup = ctx.enter_context(tc.tile_pool(name="setup", bufs=1))
    patches = ctx.enter_context(tc.tile_pool(name="patches", bufs=5))
    accs = ctx.enter_context(tc.tile_pool(name="accs", bufs=2))
    outs = ctx.enter_context(tc.tile_pool(name="outs", bufs=3))

    def st(shape, dtype=F32, tag=None):
        return setup.tile(shape, dtype, tag=tag, name=tag)

    def ts(out, in0, s1, op0, s2=None, op1=None):
        if op1 is None:
            return V.tensor_scalar(out, in0, s1, None, op0)
        return V.tensor_scalar(out, in0, s1, s2, op0, op1)

    def tt(out, in0, in1, op):
        return V.tensor_tensor(out=out, in0=in0, in1=in1, op=op)

    def b3(t, n):
        """[P,NB] tile -> broadcast AP [P,NB,n]."""
        return t[:, :].unsqueeze(2).to_broadcast([P, NB, n])

    def r3(t, n):
        """[P,NB*n] tile -> [P,NB,n] view."""
        return t[:, :].rearrange("p (b j) -> p b j", b=NB, j=n)

    # ---- load rois / offsets ----
    rt = st([P, 5], tag="rt")
    ot = st([P, 2 * NB], tag="ot")
    nc.sync.dma_start(out=rt[:, :], in_=rois_h[:, :])
    nc.sync.dma_start(out=ot[:, :], in_=offs_h[:, :])

    # ---- iotas (gpsimd int32 -> f32 copies) ----
    def iota_f32(pattern, n, tag):
        it = st([P, n], I32, tag=tag + "_i")
        nc.gpsimd.iota(it[:, :], pattern, base=0, channel_multiplier=0)
        ft = st([P, n], F32, tag=tag)
        V.tensor_copy(out=ft[:, :], in_=it[:, :])
        return ft

    iot_pw = iota_f32([[0, 7], [1, 7]], NB, "iot_pw")      # k % 7
    iot_ph = iota_f32([[1, 7], [0, 7]], NB, "iot_ph")      # k // 7
    iot_x7 = iota_f32([[0, NB], [1, NCOLS]], NB * NCOLS, "iot_x7")
    iot_y6 = iota_f32([[0, NB], [1, NROWS]], NB * NROWS, "iot_y6")
    iot_g64 = iota_f32([[0, NB], [64, NROWS]], NB * NROWS, "iot_g64")

    # ---- per-roi columns [P, 1] ----
    cols = st([P, 12], tag="cols")
    b_c = cols[:, 0:1]
    x1_c = cols[:, 1:2]
    y1_c = cols[:, 2:3]
    rw_c = cols[:, 3:4]
    rh_c = cols[:, 4:5]
    bw_c = cols[:, 5:6]
    bh_c = cols[:, 6:7]
    sw_c = cols[:, 7:8]
    sh_c = cols[:, 8:9]
    bb_c = cols[:, 9:10]   # b * 8192
    t0 = st([P, 1], tag="t0")
    t1 = st([P, 1], tag="t1")
    t2 = st([P, 1], tag="t2")

    V.tensor_copy(out=b_c, in_=rt[:, 0:1])
    ts(bb_c, b_c, 8192.0, OP.mult)

    def rnd_col(out, src_ap):
        # out = floor(src + 0.5)
        ts(t0, src_ap, 0.5, OP.add)
        ts(t1, t0, MAGIC, OP.add, MAGIC, OP.subtract)
        tt(t2, t1, t0, OP.is_gt)
        tt(out, t1, t2, OP.subtract)

    # x1 = round(r1)*S - 0.5 ; x2 = (round(r3)+1)*S - 0.5 = round(r3)*S - (0.5-S)
    rnd_col(t0, rt[:, 1:2])
    ts(x1_c, t0, S, OP.mult, 0.5, OP.subtract)
    rnd_col(t0, rt[:, 3:4])
    ts(t1, t0, S, OP.mult, 0.5 - S, OP.subtract)
    tt(rw_c, t1, x1_c, OP.subtract)
    ts(rw_c, rw_c, 0.1, OP.max)

    rnd_col(t0, rt[:, 2:3])
    ts(y1_c, t0, S, OP.mult, 0.5, OP.subtract)
    rnd_col(t0, rt[:, 4:5])
    ts(t1, t0, S, OP.mult, 0.5 - S, OP.subtract)
    tt(rh_c, t1, y1_c, OP.subtract)
    ts(rh_c, rh_c, 0.1, OP.max)

    # ---- exact f32 division by 7 (verified bit-exact over the quantized
    # domain of rw/rh = max(m/16, 0.1)): m=16x; k=floor(m/7); s=m-7k;
    # hi/lo split table of s/7; Fast2Sum(k, hi) + lo; scale by 1/16 ----
    R7 = float(np.float32(1.0) / np.float32(7.0))
    C01 = float(np.float32(0.1) / np.float32(7.0))
    dv_m = st([P, 1], tag="dv_m")
    dv_k = st([P, 1], tag="dv_k")
    dv_s = st([P, 1], tag="dv_s")
    dv_hi = st([P, 1], tag="dv_hi")
    dv_lo = st([P, 1], tag="dv_lo")
    dv_t = st([P, 1], tag="dv_t")
    dv_z = st([P, 1], tag="dv_z")
    dv_e = st([P, 1], tag="dv_e")

    def exact_div7(out, x_col):
        ts(dv_m, x_col, 16.0, OP.mult)
        ts(dv_t, dv_m, R7, OP.mult)
        ts(dv_k, dv_t, MAGIC, OP.add, MAGIC, OP.subtract)
        tt(dv_z, dv_k, dv_t, OP.is_gt)
        tt(dv_k, dv_k, dv_z, OP.subtract)
        V.scalar_tensor_tensor(out=dv_s, in0=dv_k, scalar=-7.0, in1=dv_m,
                               op0=OP.mult, op1=OP.add)
        V.memset(dv_hi[:, :], 0.0)
        V.memset(dv_lo[:, :], 0.0)
        for sv in range(1, 7):
            th = float(np.float32(sv / 7.0))
            tl = float(np.float32(sv / 7.0 - np.float32(sv / 7.0)))
            ts(dv_e, dv_s, float(sv), OP.is_equal)
            V.scalar_tensor_tensor(out=dv_hi, in0=dv_e, scalar=th, in1=dv_hi,
                                   op0=OP.mult, op1=OP.add)
            V.scalar_tensor_tensor(out=dv_lo, in0=dv_e, scalar=tl, in1=dv_lo,
                                   op0=OP.mult, op1=OP.add)
        tt(dv_t, dv_k, dv_hi, OP.add)
        tt(dv_z, dv_t, dv_k, OP.subtract)
        tt(dv_e, dv_hi, dv_z, OP.subtract)
        tt(dv_e, dv_e, dv_lo, OP.add)
        tt(dv_t, dv_t, dv_e, OP.add)
        ts(out, dv_t, 0.0625, OP.mult)
        # fix x == 0.1 exactly
        ts(dv_e, x_col, 0.1, OP.is_equal)
        ts(dv_z, out, -1.0, OP.mult, C01, OP.add)  # C01 - out
        tt(dv_z, dv_z, dv_e, OP.mult)
        tt(out, out, dv_z, OP.add)

    exact_div7(bw_c, rw_c)
    exact_div7(bh_c, rh_c)
    ts(sw_c, bw_c, 0.25, OP.mult)
    ts(sh_c, bh_c, 0.25, OP.mult)

    # ---- per-bin starts [P, NB] ----
    txt = st([P, NB], tag="txt")
    tyt = st([P, NB], tag="tyt")
    wstart = st([P, NB], tag="wstart")
    hstart = st([P, NB], tag="hstart")
    u0 = st([P, NB], tag="u0")
    ts(txt, ot[:, 0:NB], TRANS_STD, OP.mult)
    ts(tyt, ot[:, NB:2 * NB], TRANS_STD, OP.mult)
    # wstart = pw*bw + x1 + tx*rw
    ts(u0, iot_pw[:, :], bw_c, OP.mult)
    V.scalar_tensor_tensor(out=wstart[:, :], in0=txt[:, :], scalar=rw_c,
                           in1=u0[:, :], op0=OP.mult, op1=OP.add)
    ts(wstart, wstart, x1_c, OP.add)
    ts(u0, iot_ph[:, :], bh_c, OP.mult)
    V.scalar_tensor_tensor(out=hstart[:, :], in0=tyt[:, :], scalar=rh_c,
                           in1=u0[:, :], op0=OP.mult, op1=OP.add)
    ts(hstart, hstart, y1_c, OP.add)

    # ---- per-sample floor / contributions, stashed per sample ----
    def axis_samples(start_t, step_c, limit, name):
        cnt = st([P, NB], tag=f"{name}_cnt")
        wv = st([P, NB], tag=f"{name}_wv")
        vx = st([P, NB], tag=f"{name}_vx")
        wc = st([P, NB], tag=f"{name}_wc")
        g0 = st([P, NB], tag=f"{name}_g0")
        stp = st([P, 1], tag=f"{name}_stp")
        fls, c0s, c1s = [], [], []
        for s in range(SPP):
            fl = st([P, NB], tag=f"{name}_fl{s}")
            c0 = st([P, NB], tag=f"{name}_c0{s}")
            c1 = st([P, NB], tag=f"{name}_c1{s}")
            if s == 0:
                V.tensor_copy(out=wv[:, :], in_=start_t[:, :])
            else:
                ts(stp, step_c, float(s), OP.mult)
                ts(wv, start_t[:, :], stp, OP.add)
            ts(g0, wv, -0.5, OP.is_ge)
            ts(vx, wv, limit + 0.5, OP.is_le)
            tt(vx, vx, g0, OP.mult)
            if s == 0:
                V.tensor_copy(out=cnt[:, :], in_=vx[:, :])
            else:
                tt(cnt, cnt, vx, OP.add)
            ts(wc, wv, 0.0, OP.max, float(limit), OP.min)
            ts(fl, wc, MAGIC, OP.add, MAGIC, OP.subtract)
            tt(g0, fl, wc, OP.is_gt)
            tt(fl, fl, g0, OP.subtract)
            tt(wc, wc, fl, OP.subtract)          # wc now holds dx
            tt(c1, vx, wc, OP.mult)
            tt(c0, vx, c1, OP.subtract)
            fls.append(fl)
            c0s.append(c0)
            c1s.append(c1)
        return fls, c0s, c1s, cnt

    def accum_weights(wsum, eq, eqc, rel, c0, c1, iot_n, n, first):
        w3 = r3(wsum, n)
        e3 = r3(eq, n)
        ec3 = r3(eqc, n)
        tt(e3, b3(rel, n), r3(iot_n, n), OP.is_equal)
        if first:
            tt(w3, e3, b3(c0, n), OP.mult)
        else:
            tt(ec3, e3, b3(c0, n), OP.mult)
            tt(w3, w3, ec3, OP.add)
        # shifted c1 into slots 1..n-1
        cm = c1[:, :].unsqueeze(2).to_broadcast([P, NB, n - 1])
        tt(ec3[:, :, 0:n - 1], e3[:, :, 0:n - 1], cm, OP.mult)
        tt(w3[:, :, 1:n], w3[:, :, 1:n], ec3[:, :, 0:n - 1], OP.add)

    yfls, yc0s, yc1s, cnty = axis_samples(hstart, sh_c, H - 1, "y")
    xfls, xc0s, xc1s, cntx = axis_samples(wstart, sw_c, W - 1, "x")

    yb = yfls[0]                                   # row base (do not modify)

    # even-aligned x base: xe = 2*floor(xb/2); xh = floor(xb/2) (pair units)
    xb = xfls[0]
    xe = st([P, NB], tag="xe")
    xh = st([P, NB], tag="xh")
    xg = st([P, NB], tag="xg")
    ts(xh, xb[:, :], 0.5, OP.mult)
    ts(xe, xh, MAGIC, OP.add, MAGIC, OP.subtract)
    tt(xg, xe, xh, OP.is_gt)
    tt(xh, xe, xg, OP.subtract)                    # floor(xb/2)
    ts(xe, xh, 2.0, OP.mult)

    # ---- accumulate outer-product weights ----
    wy = st([P, NB * NROWS], tag="wy")
    wx = st([P, NB * NCOLS], tag="wx")
    yeq = st([P, NB * NROWS], tag="yeq")
    yeqc = st([P, NB * NROWS], tag="yeqc")
    xeq = st([P, NB * NCOLS], tag="xeq")
    xeqc = st([P, NB * NCOLS], tag="xeqc")
    yrel = st([P, NB], tag="yrel")
    xrel = st([P, NB], tag="xrel")
    for s in range(SPP):
        tt(yrel, yfls[s], yb, OP.subtract)
        accum_weights(wy, yeq, yeqc, yrel, yc0s[s], yc1s[s], iot_y6, NROWS, s == 0)
    for s in range(SPP):
        tt(xrel, xfls[s], xe, OP.subtract)
        accum_weights(wx, xeq, xeqc, xrel, xc0s[s], xc1s[s], iot_x7, NCOLS, s == 0)

    # ---- wy /= max(cnt,1) ----
    cnt = st([P, NB], tag="cnt")
    rec = st([P, NB], tag="rec")
    tt(cnt, cntx, cnty, OP.mult)
    ts(cnt, cnt, 1.0, OP.max)
    V.reciprocal(out=rec[:, :], in_=cnt[:, :])
    tt(r3(wy, NROWS), r3(wy, NROWS), b3(rec, NROWS), OP.mult)
    # bf16 copy of wy for the DVE TT-mul row (TT bf16 runs 2x)
    wy_bf = st([P, NB * NROWS], BF16, tag="wy_bf")
    V.tensor_copy(out=wy_bf[:, :], in_=wy[:, :])

    # ---- pair indices ----
    pairf = st([P, NB * NROWS], tag="pairf")
    basei = st([P, NB], tag="basei")
    V.scalar_tensor_tensor(out=basei[:, :], in0=yb[:, :], scalar=64.0,
                           in1=xh[:, :], op0=OP.mult, op1=OP.add)
    ts(basei, basei, bb_c, OP.add)
    tt(r3(pairf, NROWS), b3(basei, NROWS), r3(iot_g64, NROWS), OP.add)
    ts(pairf, pairf, 32767.0, OP.min)
    pii = st([P, NB * NROWS], I32, tag="pii")
    V.tensor_copy(out=pii[:, :], in_=pairf[:, :])
    pi16 = st([P, NB * NROWS], I16, tag="pi16")
    V.tensor_copy(out=pi16[:, :], in_=pii[:, :])

    # ---- idx layout: pos [j%16, j//16] for j = g*128 + p, replicated x8 ----
    # step 1: partition blocks 16h..16h+16 of pi16 -> partitions 0..16 of tmp
    # (engine ops can only start at partition 0/32/64/96, DMA can start anywhere)
    idxtmp = st([16, NB * NROWS * 8], I16, tag="idxtmp")  # [16, h*294+bin*6+g]
    for h in range(8):
        nc.sync.dma_start(
            out=idxtmp[0:16, h * (NB * NROWS):(h + 1) * (NB * NROWS)],
            in_=pi16[16 * h:16 * (h + 1), :],
        )
    # step 2: strided rearrange (h*294 + bin*6 + g) -> (bin*48 + g*8 + h) on DVE
    idx16 = st([P, NB * NROWS * 8], I16, tag="idx16")   # [128, 2352]
    i4 = idx16[:, :].rearrange("p (b g h) -> p b g h", b=NB, g=NROWS, h=8)
    tmp4 = bass.AP(idxtmp.tensor, idxtmp.offset,
                   [idxtmp.ap[0], [6, NB], [1, NROWS], [NB * NROWS, 8]])
    V.tensor_copy(out=i4[0:16, :, :, :], in_=tmp4)
    # step 3: replicate to all 128 partitions by doubling DMAs
    nc.sync.dma_start(out=idx16[16:32, :], in_=idx16[0:16, :])
    nc.sync.dma_start(out=idx16[32:64, :], in_=idx16[0:32, :])
    nc.sync.dma_start(out=idx16[64:128, :], in_=idx16[0:64, :])

    # ---- gather + reduce per bin ----
    feat_ap = bass.AP(feat_h, 0, [[PAIR, 32768], [1, SEG]])
    nreg = nc.gpsimd.to_reg(NROWS * P)
    for bin_ in [b for _ in range(n_repeats) for b in range(NB)]:
        patch = patches.tile([P, NROWS, SEG], BF16, tag="patch", name="patch")
        nc.gpsimd.dma_gather(
            patch[:, :, :],
            feat_ap,
            idx16[:, bin_ * 48:(bin_ + 1) * 48],
            num_idxs=NROWS * P,
            num_idxs_reg=nreg,
            elem_size=SEG,
            elem_step=PAIR,
            queue_num=bin_ % 2,
        )
        ob = outs.tile([P, C], F32, tag="ob", name="ob")
        if mac_mode == "full":
            # y-reduction: weighted rows. Muls r=0..4 on the (otherwise idle)
            # Act engine (per-partition scale AP); r=5 as a bf16 TT with a
            # broadcast weight on DVE; in-place bf16 TT tree-sum on DVE.
            pys = [accs.tile([P, SEG], BF16, tag=f"py{g}", name=f"py{g}")
                   for g in range(NROWS)]
            for g in range(5):
                wcol = wy[:, bin_ * NROWS + g:bin_ * NROWS + g + 1]
                nc.scalar.activation(pys[g][:, :], patch[:, g, :],
                                     mybir.ActivationFunctionType.Copy,
                                     bias=0.0, scale=wcol)
            wb5 = wy_bf[:, bin_ * NROWS + 5:bin_ * NROWS + 6]
            tt(pys[5], patch[:, 5, :], wb5.to_broadcast([P, SEG]), OP.mult)
            # tree: py0+=py1, py2+=py3, py4+=py5, py0+=py2, py0+=py4
            tt(pys[0], pys[0], pys[1], OP.add)
            tt(pys[2], pys[2], pys[3], OP.add)
            tt(pys[4], pys[4], pys[5], OP.add)
            tt(pys[0], pys[0], pys[2], OP.add)
            tt(pys[0], pys[0], pys[4], OP.add)
            accT = pys[0]
            # x-reduction in f32 on DVE (bf16 in0, f32 accumulate)
            for x in range(NCOLS):
                wcol = wx[:, bin_ * NCOLS + x:bin_ * NCOLS + x + 1]
                if x == 0:
                    V.tensor_scalar(ob[:, :], accT[:, 0:C], wcol, None, OP.mult)
                else:
                    V.scalar_tensor_tensor(out=ob[:, :],
                                           in0=accT[:, x * C:(x + 1) * C],
                                           scalar=wcol, in1=ob[:, :],
                                           op0=OP.mult, op1=OP.add)
        else:
            # gather-floor probe: one cheap consumer per patch
            V.tensor_scalar(ob[:, :], patch[:, 0, 0:C],
                            wy[:, bin_ * NROWS:bin_ * NROWS + 1], None, OP.mult)
        nc.sync.dma_start(out=out_h[:, bin_, :], in_=ob[:, :])


_CACHE = {}


def build_program(n_repeats: int = 1, mac_mode: str = "full"):
    key = ("nc", n_repeats, mac_mode)
    if key in _CACHE:
        return _CACHE[key]
    nc = bacc.Bacc(num_swdge_queues=2)
    feat_h = nc.declare_dram_parameter("feat", [FEAT_ELEMS + PAD_ELEMS], BF16,
                                       isOutput=False)
    rois_h = nc.declare_dram_parameter("rois", [RPC, 5], F32, isOutput=False)
    offs_h = nc.declare_dram_parameter("offs", [RPC, 2 * NB], F32, isOutput=False)
    out_h = nc.declare_dram_parameter("out", [RPC, NB, C], F32, isOutput=True)
    with tile.TileContext(nc) as tc, ExitStack() as ctx:
        _emit(ctx, tc, feat_h, rois_h, offs_h, out_h, n_repeats=n_repeats,
              mac_mode=mac_mode)
    nc.compile()
    _CACHE[key] = nc
    return nc


def make_in_maps(input, rois, offset):
    inp = np.ascontiguousarray(np.asarray(input, np.float32))
    rois = np.asarray(rois, np.float32)
    offset = np.asarray(offset, np.float32)
    featflat = np.concatenate(
        [inp.transpose(0, 2, 3, 1).reshape(-1), np.zeros(PAD_ELEMS, np.float32)]
    ).astype(ml_dtypes.bfloat16)
    in_maps = []
    for c in range(N_CORES):
        sl = slice(c * RPC, (c + 1) * RPC)
        in_maps.append({
            "feat": featflat,
            "rois": np.ascontiguousarray(rois[sl]),
            "offs": np.ascontiguousarray(offset[sl].reshape(RPC, 2 * NB)),
        })
    return in_maps


def kernel(input, rois, offset, _trace=False, _trace_kwargs=None):
    nc = build_program()
    in_maps = make_in_maps(input, rois, offset)
    res = run_bass_kernel_spmd(
        nc, in_maps, list(range(N_CORES)),
        trace=_trace, **(_trace_kwargs or {}),
    )
    outs_np = [res.results[c]["out"] for c in range(N_CORES)]
    full = np.concatenate(outs_np, axis=0)          # [1024, 49, 256]
    out = full.reshape(1024, 7, 7, 256).transpose(0, 3, 1, 2)
    if _trace:
        return np.ascontiguousarray(out), res
    return np.ascontiguousarray(out)
